# revision 1
# baseline (speedup 1.0000x reference)
"""Trainium2 Bass kernel for multi-head causal self-attention.

Problem: X [4, 2048, 1024] fp32, Wq/Wk/Wv/Wo [1024, 1024], H=16 heads, HD=64.
reference: out = softmax_causal((X@Wq) (X@Wk)^T / 8) (X@Wv) merged @ Wo.

Sharding over 8 NeuronCores: core c handles batch b = c // 2 and head group
hg = c % 2 (8 heads each). Each core computes a partial [2048, 1024] output
(its heads' contribution through Wo's row shard); the host sums the two
partials per batch (the tensor-parallel all-reduce, done during unsharding).

Per-core dataflow (bf16 operands, fp32 PSUM accumulation):
  X^T  [1024, 2048]  bf16 via XBAR DMA-transpose straight from DRAM
  Q^T,K^T [512, 2048] = (Wq chunk).T @ X^T   (partition-chunk pc = head pair)
  V    [2048, 8, 72]  = X^T.T @ Wv, heads strided, col 64 = ones
  S^T  [128k, 512q] psum = K^T.T @ Q^T  (two heads row-packed in the PE
       array; fully-causal-masked leading columns skipped on PE)
  E^T  = exp(S^T/8 [+ diag mask]) on ACT -> bf16 SBUF; masked cols zeroed
       by GpSimd memset
  O'   [72, 512] psum = [V_h | 1 | pad].T @ E^T accumulated over k-chunks;
       row 64 = softmax denominators. Copied to SBUF immediately (frees the
       PSUM bank), then reciprocal -> gpsimd partition_broadcast -> DVE
       multiply writes normalized O^T.
  OUT  [128s, 512c] = O^T.T @ Wo chunk, accumulated over 4 o-chunks
"""

import sys

for _p in ("/opt/trn_rl_repo", "/root/.axon_site/_ro/trn_rl_repo"):
    if _p not in sys.path:
        sys.path.insert(0, _p)

import ml_dtypes
import numpy as np

import concourse.bass as bass
import concourse.mybir as mybir
import concourse.tile as tile
from concourse import bacc
from concourse.bass_utils import run_bass_kernel_spmd

F32 = mybir.dt.float32
BF16 = mybir.dt.bfloat16
EXPF = mybir.ActivationFunctionType.Exp

B, S, D, H = 4, 2048, 1024, 16
HD = D // H           # 64
HL = H // 2           # 8 heads per core
DL = HL * HD          # 512 local proj width
NEG = -30000.0        # causal mask additive value (exp underflows to 0)
VW = 72               # AV lhsT width: 64 V cols + ones col + 7 pad


def build_program(s=S, d=D, hl=HL):
    dl = hl * HD
    n_st = s // 128          # s-tiles (128 rows)
    n_dc = d // 128          # d-chunks (projection contraction)
    n_pc = dl // 128         # Q^T/K^T partition chunks (= head pairs)
    n_q = s // 512           # q-chunks
    n_k = s // 128           # k-chunks
    n_cc = d // 512          # out column chunks

    nc = bacc.Bacc("TRN2", target_bir_lowering=False, debug=False)

    X = nc.dram_tensor("X", [s, d], BF16, kind="ExternalInput")
    WQ = nc.dram_tensor("WQ", [d, dl], BF16, kind="ExternalInput")
    WK = nc.dram_tensor("WK", [d, dl], BF16, kind="ExternalInput")
    WV = nc.dram_tensor("WV", [d, dl], BF16, kind="ExternalInput")
    WO = nc.dram_tensor("WO", [dl, d], BF16, kind="ExternalInput")
    OUT = nc.dram_tensor("OUT", [s, d], F32, kind="ExternalOutput")

    with tile.TileContext(nc) as tc:
        with tc.tile_pool(name="persist", bufs=1) as persist:
            # diagonal causal mask block (keep where q >= k)
            cmask = persist.tile([128, 128], F32)
            nc.gpsimd.memset(cmask[:], 0.0)
            nc.gpsimd.affine_select(
                out=cmask[:], in_=cmask[:],
                compare_op=mybir.AluOpType.is_ge, fill=NEG,
                base=0, pattern=[[1, 128]], channel_multiplier=-1,
            )

            qt = [persist.tile([128, s], BF16, name=f"qt{i}") for i in range(n_pc)]
            kt = [persist.tile([128, s], BF16, name=f"kt{i}") for i in range(n_pc)]
            vt = [persist.tile([128, hl, VW], BF16, name=f"vt{i}") for i in range(n_st)]

            _late_cm = tc.tile_pool(name="late", bufs=1)
            late = _late_cm.__enter__()
            wo = late.tile([128, n_pc, d], BF16)
            ot = [late.tile([128, s], BF16, name=f"ot{i}") for i in range(n_pc)]

            # ---- X^T + projections (interleaved by sequence block) ----
            with (
                tc.tile_pool(name="xtp", bufs=1) as xtp,
                tc.tile_pool(name="wp", bufs=1) as wp,
                tc.tile_pool(name="pps", bufs=3, space="PSUM") as pps,
            ):
                xt = [xtp.tile([128, s], BF16, name=f"xt{i}") for i in range(n_dc)]
                wq = wp.tile([128, n_dc, dl], BF16, tag="wq", name="wq")
                wk = wp.tile([128, n_dc, dl], BF16, tag="wk", name="wk")
                wv = wp.tile([128, n_dc, dl], BF16, tag="wv", name="wv")
                # wq first: the opening projection needs it; the X^T
                # transposes already dominate the ramp
                nc.sync.dma_start(
                    wq[:], WQ.ap().rearrange("(c p) m -> p c m", p=128))
                for dc in range(n_dc):
                    nc.sync.dma_start(
                        xt[dc][:], X[:, dc * 128:(dc + 1) * 128], transpose=True)
                for wsb, wdram in ((wk, WK), (wv, WV)):
                    nc.sync.dma_start(
                        wsb[:], wdram.ap().rearrange("(c p) m -> p c m", p=128))
                nc.sync.dma_start(
                    wo[:], WO.ap().rearrange("(c p) m -> p c m", p=128))
                for nq in range(s // 512):
                    for w, dst in ((wq, qt), (wk, kt)):
                        for pc in range(n_pc):
                            ps = pps.tile([128, 512], F32, tag="ps")
                            for dc in range(n_dc):
                                nc.tensor.matmul(
                                    ps[:], w[:, dc, pc * 128:(pc + 1) * 128],
                                    xt[dc][:, nq * 512:(nq + 1) * 512],
                                    start=(dc == 0), stop=(dc == n_dc - 1))
                            nc.scalar.copy(
                                dst[pc][:, nq * 512:(nq + 1) * 512], ps[:])
                    for st in range(4 * nq, 4 * nq + 4):
                        ps = pps.tile([128, dl], F32, tag="ps")
                        for dc in range(n_dc):
                            nc.tensor.matmul(
                                ps[:], xt[dc][:, st * 128:(st + 1) * 128],
                                wv[:, dc, :],
                                start=(dc == 0), stop=(dc == n_dc - 1))
                        nc.vector.memset(vt[st][:], 1.0)
                        nc.vector.tensor_copy(
                            vt[st][:, :, 0:64],
                            ps[:].rearrange("p (h e) -> p h e", h=hl))

            # ---- attention + output projection ----
            with (
                tc.tile_pool(name="work", bufs=4) as work,
                tc.tile_pool(name="norm", bufs=3) as norm_pool,
                tc.tile_pool(name="aps", bufs=4, space="PSUM") as aps,
                tc.tile_pool(name="avps", bufs=2, space="PSUM") as avps,
                tc.tile_pool(name="ops", bufs=1, space="PSUM") as ops,
            ):
                for j in range(n_q):
                    js = slice(j * 512, (j + 1) * 512)
                    for pc in range(n_pc):
                        av = [avps.tile([VW, 512], F32, tag="av", name=f"av{j}_{pc}_0"),
                              avps.tile([VW, 512], F32, tag="av", name=f"av{j}_{pc}_1")]
                        n_i = min(4 * j + 4, n_k)
                        for i in range(n_i):
                            r = i - 4 * j
                            rs = max(r, 0) * 128   # fully-masked leading cols
                            for h in (0, 1):
                                hs = slice(64 * h, 64 * h + 64)
                                stp = aps.tile([128, 512], F32, tag="stp")
                                nc.tensor.matmul(
                                    stp[:, rs:512],
                                    kt[pc][hs, i * 128:(i + 1) * 128],
                                    qt[pc][hs, j * 512 + rs:(j + 1) * 512],
                                    start=True, stop=True,
                                    tile_position=(64 * h, 0))
                                et = work.tile([128, 512], BF16, tag="et", bufs=6)
                                if r >= 0:
                                    nc.vector.tensor_add(
                                        stp[:, rs:rs + 128], stp[:, rs:rs + 128],
                                        cmask[:])
                                    if rs:
                                        nc.gpsimd.memset(et[:, 0:rs], 0.0)
                                nc.scalar.activation(
                                    et[:, rs:512], stp[:, rs:512], EXPF, scale=0.125)
                                nc.tensor.matmul(
                                    av[h][:], vt[i][:, 2 * pc + h, :], et[:],
                                    start=(i == 0), stop=(i == n_i - 1))
                        orws = []
                        dgp = norm_pool.tile(
                            [2, 512], F32, tag="dg", bufs=4, name=f"dg{j}_{pc}")
                        for h in (0, 1):
                            # free the av bank quickly: copy O' + denominators
                            orw = norm_pool.tile(
                                [VW, 512], F32, tag="orw", bufs=4,
                                name=f"orw{j}_{pc}_{h}")
                            nc.scalar.copy(orw[:], av[h][:])
                            orws.append(orw)
                            nc.sync.dma_start(
                                dgp[h:h + 1, :], orw[64:65, :])
                        rgp = norm_pool.tile(
                            [2, 512], F32, tag="rg", bufs=4, name=f"rg{j}_{pc}")
                        nc.vector.reciprocal(rgp[:], dgp[:])
                        for h in (0, 1):
                            orw = orws[h]
                            if h == 0:
                                rsrc = rgp[0:1, :]
                            else:
                                rsb = norm_pool.tile(
                                    [1, 512], F32, tag="rsb", bufs=4,
                                    name=f"rsb{j}_{pc}")
                                nc.sync.dma_start(rsb[:], rgp[1:2, :])
                                rsrc = rsb[:]
                            bc = norm_pool.tile(
                                [128, 512], F32, tag="bc", bufs=4,
                                name=f"bc{j}_{pc}_{h}")
                            nc.gpsimd.partition_broadcast(bc[:], rsrc)
                            if h == 0:
                                nc.vector.tensor_mul(
                                    ot[pc][0:64, js], orw[0:64, :], bc[0:64, :])
                            else:
                                sc = norm_pool.tile(
                                    [64, 512], BF16, tag="sc", bufs=4,
                                    name=f"sc{j}_{pc}_{h}")
                                nc.vector.tensor_mul(
                                    sc[:], orw[0:64, :], bc[0:64, :])
                                nc.sync.dma_start(ot[pc][64:128, js], sc[:])

                    last_j = j == n_q - 1 and n_pc > 1
                    for st in range(4 * j, min(4 * j + 4, n_st)):
                        for cc in range(n_cc):
                            osb = work.tile([128, 512], F32, tag="osb", bufs=2)
                            if last_j:
                                # pairs 0..n-2 accumulate and stage to SBUF
                                # while the last pair's normalization is
                                # still in flight; final pair added after
                                ps = ops.tile([128, 512], F32, tag="outp", bufs=2)
                                for pc in range(n_pc - 1):
                                    nc.tensor.matmul(
                                        ps[:], ot[pc][:, st * 128:(st + 1) * 128],
                                        wo[:, pc, cc * 512:(cc + 1) * 512],
                                        start=(pc == 0), stop=(pc == n_pc - 2))
                                nc.vector.tensor_copy(osb[:], ps[:])
                                psb = ops.tile([128, 512], F32, tag="outp", bufs=2)
                                nc.tensor.matmul(
                                    psb[:], ot[n_pc - 1][:, st * 128:(st + 1) * 128],
                                    wo[:, n_pc - 1, cc * 512:(cc + 1) * 512],
                                    start=True, stop=True)
                                nc.vector.tensor_add(osb[:], osb[:], psb[:])
                            else:
                                ps = ops.tile([128, 512], F32, tag="outp", bufs=2)
                                for pc in range(n_pc):
                                    nc.tensor.matmul(
                                        ps[:], ot[pc][:, st * 128:(st + 1) * 128],
                                        wo[:, pc, cc * 512:(cc + 1) * 512],
                                        start=(pc == 0), stop=(pc == n_pc - 1))
                                nc.vector.tensor_copy(osb[:], ps[:])
                            nc.sync.dma_start(
                                OUT[st * 128:(st + 1) * 128,
                                    cc * 512:(cc + 1) * 512],
                                osb[:])

            _late_cm.__exit__(None, None, None)

    nc.compile()
    return nc


_NC_CACHE = {}


def _get_program():
    key = (S, D, HL)
    if key not in _NC_CACHE:
        _NC_CACHE[key] = build_program()
    return _NC_CACHE[key]


def _bf16(a):
    return np.ascontiguousarray(a.astype(ml_dtypes.bfloat16))


def make_in_maps(X, Wq, Wk, Wv, Wo):
    in_maps = []
    for c in range(8):
        b, hg = c // 2, c % 2
        cs = slice(hg * DL, hg * DL + DL)
        in_maps.append({
            "X": _bf16(X[b]),
            "WQ": _bf16(Wq[:, cs]),
            "WK": _bf16(Wk[:, cs]),
            "WV": _bf16(Wv[:, cs]),
            "WO": _bf16(Wo[cs, :]),
        })
    return in_maps


def gather_out(results):
    out = np.empty((B, S, D), dtype=np.float32)
    for b in range(B):
        out[b] = results[2 * b]["OUT"] + results[2 * b + 1]["OUT"]
    return out


def kernel(X, Wq, Wk, Wv, Wo):
    X = np.asarray(X, dtype=np.float32)
    Wq = np.asarray(Wq, dtype=np.float32)
    Wk = np.asarray(Wk, dtype=np.float32)
    Wv = np.asarray(Wv, dtype=np.float32)
    Wo = np.asarray(Wo, dtype=np.float32)

    nc = _get_program()
    in_maps = make_in_maps(X, Wq, Wk, Wv, Wo)
    res = run_bass_kernel_spmd(nc, in_maps, list(range(8)), trace=False)
    return gather_out(res.results)


if __name__ == "__main__":
    rng = np.random.default_rng(0)
    scale = 1.0 / np.sqrt(D)
    inputs = {
        "X": rng.standard_normal((B, S, D), dtype=np.float32),
        "Wq": rng.standard_normal((D, D), dtype=np.float32) * scale,
        "Wk": rng.standard_normal((D, D), dtype=np.float32) * scale,
        "Wv": rng.standard_normal((D, D), dtype=np.float32) * scale,
        "Wo": rng.standard_normal((D, D), dtype=np.float32) * scale,
    }
    out = kernel(**inputs)
    print("kernel output shape:", out.shape)



# revision 10
# speedup vs baseline: 1.3227x; 1.3227x over previous
"""Trainium2 Bass kernel for multi-head causal self-attention.

Problem: X [4, 2048, 1024] fp32, Wq/Wk/Wv/Wo [1024, 1024], H=16 heads, HD=64.
reference: out = softmax_causal((X@Wq) (X@Wk)^T / 8) (X@Wv) merged @ Wo.

Sharding over 8 NeuronCores: core c handles batch b = c // 2 and head group
hg = c % 2 (8 heads each). Each core computes a partial [2048, 1024] output
(its heads' contribution through Wo's row shard); the host sums the two
partials per batch (the tensor-parallel all-reduce, done during unsharding).

v2 dataflow (bf16 operands, fp32 PSUM accumulation), software-pipelined so
the ACT-engine exp stream hides behind PE matmuls:
  X^T  [1024, 2048] bf16 via XBAR DMA-transpose (split across sync+scalar qs)
  Q^T,K^T [512, 2048] = (W chunk).T @ X^T    per q-chunk, interleaved into
       the previous chunk's attention as PE filler
  V    [2048, 8, 72]  = X^T.T @ Wv, heads strided, col 64 = ones
  S^T  pair [128k, 2, 512q] psum = K^T.T @ Q^T, both heads of a pc emitted
       back-to-back with tile_position (0,0)/(64,0) -> concurrent row-tiles;
       fully-masked leading columns skipped
  E^T  = exp(S^T/8 [+ diag mask]) on ACT, one instruction per k-tile covering
       both heads' psum banks -> bf16 SBUF
  O'   [72, 2, 512] psum = [V_h | 1 | pad].T @ E^T accumulated over k-tiles,
       masked columns never written (has_written overwrite semantics);
       row 64 = softmax denominators
  norm reciprocal_approx_fast on psum row 64 -> gpsimd partition_broadcast
       -> DVE multiply (psum x bcast) writes normalized O^T
  OUT  [128s, 512c] = O^T.T @ Wo chunk over 4 pc; bf16 out, host sums pairs
"""

import sys

for _p in ("/opt/trn_rl_repo", "/root/.axon_site/_ro/trn_rl_repo"):
    if _p not in sys.path:
        sys.path.insert(0, _p)

import ml_dtypes
import numpy as np

import concourse.bass as bass
import concourse.mybir as mybir
import concourse.tile as tile
from concourse import bacc
from concourse.bass_utils import run_bass_kernel_spmd

F32 = mybir.dt.float32
BF16 = mybir.dt.bfloat16
EXPF = mybir.ActivationFunctionType.Exp

B, S, D, H = 4, 2048, 1024, 16
HD = D // H           # 64
HL = H // 2           # 8 heads per core
DL = HL * HD          # 512 local proj width
NEG = -30000.0        # causal mask additive value (exp underflows to 0)
VW = 72               # AV lhsT width: 64 V cols + ones col + 7 pad
INTERLEAVE = True     # software-pipeline projections into attention


def build_program(s=S, d=D, hl=HL):
    dl = hl * HD
    n_st = s // 128          # s-tiles (128 rows)
    n_dc = d // 128          # d-chunks (projection contraction)
    n_pc = dl // 128         # Q^T/K^T partition chunks (= head pairs)
    n_q = s // 512           # q-chunks
    n_cc = d // 512          # out column chunks

    nc = bacc.Bacc("TRN2", target_bir_lowering=False, debug=False)

    X = nc.dram_tensor("X", [s, d], BF16, kind="ExternalInput")
    WQ = nc.dram_tensor("WQ", [d, dl], BF16, kind="ExternalInput")
    WK = nc.dram_tensor("WK", [d, dl], BF16, kind="ExternalInput")
    WV = nc.dram_tensor("WV", [d, dl], BF16, kind="ExternalInput")
    WO = nc.dram_tensor("WO", [dl, d], BF16, kind="ExternalInput")
    OUT = nc.dram_tensor("OUT", [s, d], BF16, kind="ExternalOutput")

    with tile.TileContext(nc) as tc:
        with (
            tc.tile_pool(name="persist", bufs=1) as persist,
            tc.tile_pool(name="fillps", bufs=2, space="PSUM") as fillps,
            tc.tile_pool(name="stps", bufs=2, space="PSUM") as stps,
            tc.tile_pool(name="avps", bufs=1, space="PSUM") as avps,
            tc.tile_pool(name="work", bufs=1) as work,
        ):
            # diagonal causal mask block (keep where q >= k)
            cmask = persist.tile([128, 128], F32)
            nc.gpsimd.memset(cmask[:], 0.0)
            nc.gpsimd.affine_select(
                out=cmask[:], in_=cmask[:],
                compare_op=mybir.AluOpType.is_ge, fill=NEG,
                base=0, pattern=[[1, 128]], channel_multiplier=-1,
            )

            qt = [persist.tile([128, s], BF16, name=f"qt{i}") for i in range(n_pc)]
            kt = [persist.tile([128, s], BF16, name=f"kt{i}") for i in range(n_pc)]
            vt = [persist.tile([128, hl, VW], BF16, name=f"vt{i}") for i in range(n_st)]
            ot = [persist.tile([128, s], BF16, name=f"ot{i}") for i in range(n_pc)]
            xt = persist.tile([128, n_dc, s], BF16, name="xt")
            wq = persist.tile([128, n_dc, dl], BF16, name="wq")
            wk = persist.tile([128, n_dc, dl], BF16, name="wk")
            wv = persist.tile([128, n_dc, dl], BF16, name="wv")
            wo = persist.tile([128, n_pc, d], BF16, name="wo")

            # ---- ramp: weight + X^T DMAs (transposes split across queues) ----
            nc.sync.dma_start(
                wq[:], WQ.ap().rearrange("(c p) m -> p c m", p=128))
            # single XBAR transpose: concurrent/multi-queue transposes
            # corrupt each other (shared xbar state) -- one instruction is
            # safe and saves 7 dispatch slots on the sync queue
            nc.sync.dma_start(xt[:], X[:, :], transpose=True)
            nc.sync.dma_start(
                wk[:], WK.ap().rearrange("(c p) m -> p c m", p=128))
            nc.sync.dma_start(
                wv[:], WV.ap().rearrange("(c p) m -> p c m", p=128))
            nc.sync.dma_start(
                wo[:], WO.ap().rearrange("(c p) m -> p c m", p=128))
            # ones / pad columns of V tiles, set once
            for st in range(n_st):
                nc.gpsimd.memset(vt[st][:, :, HD:VW], 1.0)

            # ---------- filler units (each ~0.4-1.7us of PE work) ----------
            def qk_unit(jn, pc, w, dst):
                def run():
                    ps = fillps.tile([128, 512], F32, tag="fill", name="psqk")
                    for dc in range(n_dc):
                        nc.tensor.matmul(
                            ps[:], w[:, dc, pc * 128:(pc + 1) * 128],
                            xt[:, dc, jn * 512:(jn + 1) * 512],
                            start=(dc == 0), stop=(dc == n_dc - 1))
                    nc.vector.tensor_copy(
                        dst[pc][:, jn * 512:(jn + 1) * 512], ps[:])
                return run

            def v_unit(st):
                def run():
                    ps = fillps.tile([128, dl], F32, tag="fill", name="psv")
                    for dc in range(n_dc):
                        nc.tensor.matmul(
                            ps[:], xt[:, dc, st * 128:(st + 1) * 128],
                            wv[:, dc, :],
                            start=(dc == 0), stop=(dc == n_dc - 1))
                    nc.vector.tensor_copy(
                        vt[st][:, :, 0:HD],
                        ps[:].rearrange("p (h e) -> p h e", h=hl))
                return run

            def proj_units(jn):
                us = []
                for pc in range(n_pc):
                    us.append(qk_unit(jn, pc, wq, qt))
                for pc in range(n_pc):
                    us.append(qk_unit(jn, pc, wk, kt))
                for st in range(4 * jn, 4 * jn + 4):
                    us.append(v_unit(st))
                return us

            def outproj_unit(jo, st, cc):
                def run():
                    ps = fillps.tile([128, 512], F32, tag="fill", name="psop")
                    for pc in range(n_pc):
                        nc.tensor.matmul(
                            ps[:], ot[pc][:, st * 128:(st + 1) * 128],
                            wo[:, pc, cc * 512:(cc + 1) * 512],
                            start=(pc == 0), stop=(pc == n_pc - 1))
                    osb = work.tile([128, 512], BF16, tag="osb", bufs=3,
                                    name="osb")
                    nc.vector.tensor_copy(osb[:], ps[:])
                    nc.sync.dma_start(
                        OUT[st * 128:(st + 1) * 128,
                            cc * 512:(cc + 1) * 512], osb[:])
                return run

            def outproj_units(jo):
                return [outproj_unit(jo, st, cc)
                        for st in range(4 * jo, 4 * jo + 4)
                        for cc in range(n_cc)]

            # last q-chunk: accumulate pc 0..2 early (partial), add pc 3 after
            # its normalization lands, so the tail only waits on 1-MM chains
            lj_osb = {}

            def outproj_partial_unit(st, cc):
                def run():
                    ps = fillps.tile([128, 512], F32, tag="fill", name="psp")
                    for pc in range(n_pc - 1):
                        nc.tensor.matmul(
                            ps[:], ot[pc][:, st * 128:(st + 1) * 128],
                            wo[:, pc, cc * 512:(cc + 1) * 512],
                            start=(pc == 0), stop=(pc == n_pc - 2))
                    osb = work.tile([128, 512], F32, tag="osb3", bufs=8,
                                    name="osb3")
                    nc.vector.tensor_copy(osb[:], ps[:])
                    lj_osb[(st, cc)] = osb
                return run

            def outproj_final_unit(st, cc):
                def run():
                    ps = fillps.tile([128, 512], F32, tag="fill", name="psf")
                    nc.tensor.matmul(
                        ps[:], ot[n_pc - 1][:, st * 128:(st + 1) * 128],
                        wo[:, n_pc - 1, cc * 512:(cc + 1) * 512],
                        start=True, stop=True)
                    osb = lj_osb[(st, cc)]
                    ofin = work.tile([128, 512], BF16, tag="ofin", bufs=3,
                                     name="ofin")
                    nc.vector.tensor_add(ofin[:], osb[:], ps[:])
                    nc.sync.dma_start(
                        OUT[st * 128:(st + 1) * 128,
                            cc * 512:(cc + 1) * 512], ofin[:])
                return run

            # ---- chunk-0 projections run un-interleaved (nothing to hide) ----
            for u in proj_units(0):
                u()

            # ---- attention per q-chunk with PE filler interleave ----
            def attention(j, fill, front, late=()):
                """fill: filler units spread over this chunk's iterations.
                front: units pulled 1/iteration at the start (KV deps).
                late: units pulled only during the last pc's iterations
                (they depend on this chunk's earlier pc results)."""
                fill = list(fill)
                late = list(late)
                n_iter = n_pc * 4 * (j + 1)
                n_last = 4 * (j + 1)
                spread = max(len(fill) - front, 0)
                credit = 0.0
                lcredit = 0.0
                pulled = 0
                lpulled = 0

                def pull(in_last_pc):
                    nonlocal credit, lcredit, pulled, lpulled
                    if in_last_pc and late:
                        lcredit += len(late) / n_last
                        while lcredit >= 1.0 and lpulled < len(late):
                            late[lpulled]()
                            lpulled += 1
                            lcredit -= 1.0
                    if pulled < front and pulled < len(fill):
                        fill[pulled]()
                        pulled += 1
                        return
                    credit += spread / max(n_iter - front, 1)
                    while credit >= 1.0 and pulled < len(fill):
                        fill[pulled]()
                        pulled += 1
                        credit -= 1.0

                js = slice(j * 512, (j + 1) * 512)
                n_i = 4 * (j + 1)
                for pc in range(n_pc):
                    av = avps.tile([VW, 2, 512], F32, tag="av",
                                   name=f"av{j}_{pc}")
                    # iterate non-diagonal k-tiles first: they only need
                    # this chunk's Q plus older K/V, so KV filler for this
                    # chunk can still be in flight
                    order = list(range(4 * j)) + list(range(4 * j, n_i))
                    first = order[0]
                    last = order[-1]
                    for i in order:
                        r = i - 4 * j
                        rs = max(r, 0) * 128
                        stp = stps.tile([128, 2, 512], F32, tag="stp",
                                        name=f"stp{j}_{pc}_{i}")
                        for h in (0, 1):
                            hs = slice(64 * h, 64 * h + 64)
                            nc.tensor.matmul(
                                stp[:, h, rs:512],
                                kt[pc][hs, i * 128:(i + 1) * 128],
                                qt[pc][hs, j * 512 + rs:(j + 1) * 512],
                                start=True, stop=True,
                                tile_position=(64 * h, 0))
                        if r >= 0:
                            for h in (0, 1):
                                nc.vector.tensor_add(
                                    stp[:, h, rs:rs + 128],
                                    stp[:, h, rs:rs + 128], cmask[:])
                        et = work.tile([128, 2, 512], BF16, tag="et", bufs=4,
                                       name=f"et{j}_{pc}_{i}")
                        nc.scalar.activation(
                            et[:, :, rs:512], stp[:, :, rs:512], EXPF,
                            scale=0.125)
                        pull(pc == n_pc - 1)
                        for h in (0, 1):
                            nc.tensor.matmul(
                                av[:, h, rs:512], vt[i][:, 2 * pc + h, :],
                                et[:, h, rs:512],
                                start=(i == first), stop=(i == last))
                    # softmax normalization straight off the AV psum.
                    # tensor_copy shifts the denominator row to partition 0
                    # (custom DVE ops require base-partition-0 APs)
                    den = work.tile([1, 2, 512], F32, tag="den", bufs=2,
                                    name=f"den{j}_{pc}")
                    nc.vector.tensor_copy(den[:], av[64:65, :, :])
                    rinv = work.tile([1, 2, 512], F32, tag="rinv", bufs=2,
                                     name=f"rinv{j}_{pc}")
                    nc.vector.reciprocal_approx_fast(rinv[:], den[:])
                    for h in (0, 1):
                        bc = work.tile([64, 512], F32, tag="bc", bufs=2,
                                       name=f"bc{j}_{pc}_{h}")
                        nc.gpsimd.partition_broadcast(bc[:], rinv[0:1, h, :])
                        if h == 0:
                            nc.vector.tensor_mul(
                                ot[pc][0:64, js], av[0:64, h, :], bc[:])
                        else:
                            sc = work.tile([64, 512], BF16, tag="sc", bufs=2,
                                           name=f"sc{j}_{pc}")
                            nc.vector.tensor_mul(
                                sc[:], av[0:64, h, :], bc[:])
                            nc.gpsimd.dma_start(ot[pc][64:128, js], sc[:])
                # drain remaining filler
                while pulled < len(fill):
                    fill[pulled]()
                    pulled += 1
                while lpulled < len(late):
                    late[lpulled]()
                    lpulled += 1

            if INTERLEAVE:
                attention(0, proj_units(1), front=0)
                attention(1, proj_units(2), front=0)
                attention(2, (proj_units(3)[:4]           # Q(3)
                              + outproj_units(0) + outproj_units(1)),
                          front=0)
                fill3 = (proj_units(3)[4:]                # K(3), V(3)
                         + outproj_units(2))
                late3 = [outproj_partial_unit(st, cc)
                         for st in range(12, 16) for cc in range(n_cc)]
                attention(3, fill3, front=8, late=late3)
                for st in range(12, 16):
                    for cc in range(n_cc):
                        outproj_final_unit(st, cc)()
            else:
                attention(0, [], front=0)
                for u in proj_units(1):
                    u()
                attention(1, [], front=0)
                for u in proj_units(2):
                    u()
                attention(2, [], front=0)
                for u in proj_units(3):
                    u()
                attention(3, [], front=0)
                for jo in range(4):
                    for u in outproj_units(jo):
                        u()

    nc.compile()
    return nc


_NC_CACHE = {}


def _get_program():
    key = (S, D, HL)
    if key not in _NC_CACHE:
        _NC_CACHE[key] = build_program()
    return _NC_CACHE[key]


def _bf16(a):
    return np.ascontiguousarray(a.astype(ml_dtypes.bfloat16))


def make_in_maps(X, Wq, Wk, Wv, Wo):
    in_maps = []
    for c in range(8):
        b, hg = c // 2, c % 2
        cs = slice(hg * DL, hg * DL + DL)
        in_maps.append({
            "X": _bf16(X[b]),
            "WQ": _bf16(Wq[:, cs]),
            "WK": _bf16(Wk[:, cs]),
            "WV": _bf16(Wv[:, cs]),
            "WO": _bf16(Wo[cs, :]),
        })
    return in_maps


def gather_out(results):
    out = np.empty((B, S, D), dtype=np.float32)
    for b in range(B):
        out[b] = (results[2 * b]["OUT"].astype(np.float32)
                  + results[2 * b + 1]["OUT"].astype(np.float32))
    return out


def kernel(X, Wq, Wk, Wv, Wo):
    X = np.asarray(X, dtype=np.float32)
    Wq = np.asarray(Wq, dtype=np.float32)
    Wk = np.asarray(Wk, dtype=np.float32)
    Wv = np.asarray(Wv, dtype=np.float32)
    Wo = np.asarray(Wo, dtype=np.float32)

    nc = _get_program()
    in_maps = make_in_maps(X, Wq, Wk, Wv, Wo)
    res = run_bass_kernel_spmd(nc, in_maps, list(range(8)), trace=False)
    return gather_out(res.results)


if __name__ == "__main__":
    rng = np.random.default_rng(0)
    scale = 1.0 / np.sqrt(D)
    inputs = {
        "X": rng.standard_normal((B, S, D), dtype=np.float32),
        "Wq": rng.standard_normal((D, D), dtype=np.float32) * scale,
        "Wk": rng.standard_normal((D, D), dtype=np.float32) * scale,
        "Wv": rng.standard_normal((D, D), dtype=np.float32) * scale,
        "Wo": rng.standard_normal((D, D), dtype=np.float32) * scale,
    }
    out = kernel(**inputs)
    print("kernel output shape:", out.shape)
